# revision 13
# baseline (speedup 1.0000x reference)
"""Trainium2 Bass kernel for nn_ClassificationHead.

Reference computation (B=64, S=512, H=1024, L=30):
    ss = argmax(sub_mask == 7);  se = argmax(sub_mask == 8)
    os = argmax(obj_mask == 9);  oe = argmax(obj_mask == 10)
    ent = (2*f[b,ss] + 2*f[b,se] + f[b,os] + f[b,oe]) / 6          # [B, H]
    h   = gelu(ent @ W1.T + b1)                                     # [B, H]
    out = h @ W2.T + b2                                             # [B, L]

Strategy: data-parallel over 8 NeuronCores (8 samples each), MLP weights
replicated. Per core everything is done on-device:
  - marker indices via iota/is_equal/reduce on DVE,
  - row gather via indirect DMA,
  - entity pooling + transpose fused into one PE matmul per k-chunk
    against a constant selection matrix,
  - fp32 matmuls with batch as the stationary operand (8-column weight
    loads are nearly free), W1.T streamed in 8 k-chunks so the PE rides
    the DMA,
  - biases folded in as K=1 accumulating matmuls against a ones-row.
Weights are passed pre-transposed (layout choice on the host).
"""
import numpy as np

import concourse.bass as bass
import concourse.tile as tile
from concourse import bacc, mybir
from concourse import bass_utils
from concourse.masks import make_identity

B, S, H, L = 64, 512, 1024, 30
N_CORES = 8
BP = B // N_CORES          # samples per core
KC = H // 128              # k-chunks of 128
F32 = mybir.dt.float32
I32 = mybir.dt.int32

_cache = {}


def _build(enable_asserts=False, gelu="exact", use_indirect=True,
           idx_mode="compute", use_bias_mm=True):
    nc = bacc.Bacc("TRN2", target_bir_lowering=False, debug=False,
                   enable_asserts=enable_asserts, num_devices=N_CORES)
    feat = nc.dram_tensor("feat", [BP * S, H], F32, kind="ExternalInput").ap()
    masks = nc.dram_tensor("masks", [2 * BP, S], I32, kind="ExternalInput").ap()
    w1t = nc.dram_tensor("w1t", [H, H], F32, kind="ExternalInput").ap()
    b1v = nc.dram_tensor("b1v", [1, H], F32, kind="ExternalInput").ap()
    w2t = nc.dram_tensor("w2t", [H, L], F32, kind="ExternalInput").ap()
    b2v = nc.dram_tensor("b2v", [1, L], F32, kind="ExternalInput").ap()
    wsel = nc.dram_tensor("wsel", [4 * BP, BP], F32, kind="ExternalInput").ap()
    mvals = nc.dram_tensor("mvals", [4 * BP, 1], I32, kind="ExternalInput").ap()
    boffv = nc.dram_tensor("boffv", [4 * BP, 1], F32, kind="ExternalInput").ap()
    out = nc.dram_tensor("out", [BP, L], F32, kind="ExternalOutput").ap()

    P4 = 4 * BP            # 32 partitions: [marker, sample]
    w1r = w1t.rearrange("(c p) j -> c p j", p=128)
    w2r = w2t.rearrange("(c p) l -> p c l", p=128)

    from contextlib import ExitStack
    with tile.TileContext(nc) as tc, ExitStack() as ctx:
        sb_pool = ctx.enter_context(tc.tile_pool(name="persist", bufs=1))
        psum_pool = ctx.enter_context(
            tc.tile_pool(name="psum", bufs=1, space="PSUM"))

        def mktile(name, shape, dtype, space="SBUF"):
            pool = psum_pool if space == "PSUM" else sb_pool
            return pool.tile(shape, dtype, name=name)

        # ---- persistent tiles -------------------------------------------
        masks_sb = mktile("masks_sb", [P4, S], I32)
        wsel_sb = mktile("wsel_sb", [P4, BP], F32)
        mvals_sb = mktile("mvals_sb", [P4, 1], I32)
        boff_sb = mktile("boff_sb", [P4, 1], F32)
        iota_sb = mktile("iota_sb", [P4, S], F32)
        eq_sb = mktile("eq_sb", [P4, S], F32)
        scr_sb = mktile("scr_sb", [P4, S], F32)
        idxf_sb = mktile("idxf_sb", [P4, 1], F32)
        idxi_sb = mktile("idxi_sb", [P4, 1], I32)
        gath_sb = mktile("gath_sb", [P4, H], F32)
        entT_sb = mktile("entT_sb", [128, KC * BP], F32)
        h_sb = mktile("h_sb", [BP, H], F32)
        hT_sb = mktile("hT_sb", [128, KC * BP], F32)
        ones_sb = mktile("ones_sb", [1, BP], F32)
        i8_sb = mktile("i8_sb", [BP, BP], F32)
        b1_sb = mktile("b1_sb", [1, H], F32)
        b2_sb = mktile("b2_sb", [1, L], F32)
        w2_sb = mktile("w2_sb", [128, KC, L], F32)
        out_sb = mktile("out_sb", [BP, L], F32)

        ps_ent = mktile("ps_ent", [128, KC * BP], F32, space="PSUM")
        ps_h0 = mktile("ps_h0", [BP, 512], F32, space="PSUM")
        ps_h1 = mktile("ps_h1", [BP, 512], F32, space="PSUM")
        ps_hT = mktile("ps_hT", [128, KC * BP], F32, space="PSUM")
        ps_o = mktile("ps_o", [BP, L], F32, space="PSUM")
        ps_h = [ps_h0, ps_h1]

        # ---- critical-path head: masks + small consts -------------------
        # masks tile layout: rows 0-7 sub, 8-15 sub, 16-23 obj, 24-31 obj
        nc.sync.dma_start(masks_sb[0:BP, :], masks[0:BP, :])
        nc.sync.dma_start(masks_sb[BP:2 * BP, :], masks[0:BP, :])
        nc.sync.dma_start(masks_sb[2 * BP:3 * BP, :], masks[BP:2 * BP, :])
        nc.sync.dma_start(masks_sb[3 * BP:4 * BP, :], masks[BP:2 * BP, :])
        nc.sync.dma_start(mvals_sb[:], mvals)
        nc.sync.dma_start(boff_sb[:], boffv)
        nc.sync.dma_start(wsel_sb[:], wsel)

        # setup on gpsimd (off the DVE/ACT/PE critical path)
        nc.gpsimd.iota(iota_sb[:], pattern=[[1, S]], base=0,
                       channel_multiplier=0,
                       allow_small_or_imprecise_dtypes=True)
        nc.gpsimd.memset(ones_sb[:], 1.0)
        make_identity(nc, i8_sb[:])

        # ---- bulk weight loads ------------------------------------------
        with tc.tile_pool(name="w1pool", bufs=1) as w1pool:
            w1_tiles = []
            for c in range(KC):
                t = w1pool.tile([128, H], F32, name=f"w1c{c}")
                nc.sync.dma_start(t[:], w1r[c])
                w1_tiles.append(t)
            nc.sync.dma_start(w2_sb[:], w2r)
            nc.sync.dma_start(b1_sb[:], b1v)
            nc.sync.dma_start(b2_sb[:], b2v)

            # ---- marker indices ----------------------------------------
            if idx_mode in ("compute", "c1", "c2"):
                # eq[p, s] = (mask[p, s] == mval[p])
                nc.vector.tensor_tensor(
                    out=eq_sb[:], in0=masks_sb[:],
                    in1=mvals_sb[:, :1].to_broadcast([P4, S]),
                    op=mybir.AluOpType.is_equal)
            if idx_mode in ("compute", "c2"):
                # idxf[p] = sum_s eq[p, s] * s  (exactly one match per row)
                # (tensor_tensor_reduce wedges the device on this runtime —
                #  use separate mult + reduce)
                nc.vector.tensor_tensor(
                    out=scr_sb[:], in0=eq_sb[:], in1=iota_sb[:],
                    op=mybir.AluOpType.mult)
                nc.vector.tensor_reduce(
                    out=idxf_sb[:], in_=scr_sb[:],
                    axis=mybir.AxisListType.X, op=mybir.AluOpType.add)
            if idx_mode == "compute":
                # row index into feat: idx + 512*(p % 8); cast to int32
                nc.vector.tensor_tensor(
                    out=idxi_sb[:], in0=idxf_sb[:], in1=boff_sb[:],
                    op=mybir.AluOpType.add)
            else:  # debug: fixed indices 0..31 via iota
                nc.gpsimd.iota(idxi_sb[:], pattern=[[0, 1]], base=0,
                               channel_multiplier=1)

            # ---- gather the 32 marker rows -----------------------------
            if use_indirect:
                nc.gpsimd.indirect_dma_start(
                    out=gath_sb[:], out_offset=None,
                    in_=feat,
                    in_offset=bass.IndirectOffsetOnAxis(
                        ap=idxi_sb[:, :1], axis=0))
            else:  # debug: direct rows 0..31
                nc.sync.dma_start(gath_sb[:], feat[0:P4, :])

            # ---- entity pooling + transpose in one matmul per chunk ----
            # entT[k, b] = sum_p gath[p, k] * wsel[p, b]
            for c in range(KC):
                nc.tensor.matmul(
                    out=ps_ent[:, c * BP:(c + 1) * BP],
                    lhsT=gath_sb[:, c * 128:(c + 1) * 128],
                    rhs=wsel_sb[:], start=True, stop=True)
            nc.vector.tensor_copy(entT_sb[:], ps_ent[:])

            # ---- matmul1: h_pre[b, j] = ent @ W1.T + b1 ----------------
            for c in range(KC):
                for j in range(2):
                    nc.tensor.matmul(
                        out=ps_h[j][:],
                        lhsT=entT_sb[:, c * BP:(c + 1) * BP],
                        rhs=w1_tiles[c][:, j * 512:(j + 1) * 512],
                        start=(c == 0),
                        stop=(c == KC - 1 and not use_bias_mm))
            for j in range(2):
                if use_bias_mm:
                    nc.tensor.matmul(
                        out=ps_h[j][:], lhsT=ones_sb[:1, :],
                        rhs=b1_sb[:1, j * 512:(j + 1) * 512],
                        start=False, stop=True)
                # ---- gelu (exact erf-based on HW) ----------------------
                hsl = h_sb[:, j * 512:(j + 1) * 512]
                if gelu == "exact":
                    nc.scalar.activation(
                        hsl, ps_h[j][:], mybir.ActivationFunctionType.Gelu)
                else:
                    # CoreSim lacks Gelu: x * sigmoid(1.702 x) stand-in
                    sig_sb = mktile(f"sig_sb{j}", [BP, 512], F32)
                    hx_sb = mktile(f"hx_sb{j}", [BP, 512], F32)
                    nc.scalar.activation(
                        sig_sb[:], ps_h[j][:],
                        mybir.ActivationFunctionType.Sigmoid, scale=1.702)
                    nc.vector.tensor_copy(hx_sb[:], ps_h[j][:])
                    nc.vector.tensor_tensor(
                        out=hsl, in0=hx_sb[:], in1=sig_sb[:],
                        op=mybir.AluOpType.mult)

            # ---- transpose h -------------------------------------------
            for c in range(KC):
                nc.tensor.matmul(
                    out=ps_hT[:, c * BP:(c + 1) * BP],
                    lhsT=h_sb[:, c * 128:(c + 1) * 128],
                    rhs=i8_sb[:], start=True, stop=True)
            nc.vector.tensor_copy(hT_sb[:], ps_hT[:])

            # ---- matmul2: out[b, l] = h @ W2.T + b2 --------------------
            for c in range(KC):
                nc.tensor.matmul(
                    out=ps_o[:],
                    lhsT=hT_sb[:, c * BP:(c + 1) * BP],
                    rhs=w2_sb[:, c, :], start=(c == 0),
                    stop=(c == KC - 1 and not use_bias_mm))
            if use_bias_mm:
                nc.tensor.matmul(
                    out=ps_o[:], lhsT=ones_sb[:1, :], rhs=b2_sb[:1, :],
                    start=False, stop=True)
            nc.vector.tensor_copy(out_sb[:], ps_o[:])
            nc.sync.dma_start(out, out_sb[:])

    nc.compile()
    return nc


def _host_inputs(features, sub_mask, obj_mask, W1, b1, W2, b2):
    """Per-core input dicts. Host work is layout only (shard/transpose/consts)."""
    w1t = np.ascontiguousarray(W1.T)
    w2t = np.ascontiguousarray(W2.T)
    b1v = np.ascontiguousarray(b1.reshape(1, H))
    b2v = np.ascontiguousarray(b2.reshape(1, L))
    # selection matrix: wsel[m*BP + b, b] = weight(m); weights (2,2,1,1)/6
    wsel = np.zeros((4 * BP, BP), np.float32)
    wm = np.array([2.0, 2.0, 1.0, 1.0], np.float32) / 6.0
    for m in range(4):
        for b in range(BP):
            wsel[m * BP + b, b] = wm[m]
    mvals = np.array([7] * BP + [8] * BP + [9] * BP + [10] * BP,
                     np.int32).reshape(4 * BP, 1)
    boffv = (np.tile(np.arange(BP, dtype=np.float32), 4) * S).reshape(4 * BP, 1)

    in_maps = []
    for core in range(N_CORES):
        sl = slice(core * BP, (core + 1) * BP)
        in_maps.append({
            "feat": np.ascontiguousarray(
                features[sl].reshape(BP * S, H).astype(np.float32)),
            "masks": np.ascontiguousarray(np.concatenate(
                [sub_mask[sl], obj_mask[sl]]).astype(np.int32)),
            "w1t": w1t, "b1v": b1v, "w2t": w2t, "b2v": b2v,
            "wsel": wsel, "mvals": mvals, "boffv": boffv,
        })
    return in_maps


def kernel(features, sub_mask, obj_mask, W1, b1, W2, b2, _trace=False):
    features = np.asarray(features)
    sub_mask = np.asarray(sub_mask)
    obj_mask = np.asarray(obj_mask)
    W1 = np.asarray(W1, np.float32)
    b1 = np.asarray(b1, np.float32)
    W2 = np.asarray(W2, np.float32)
    b2 = np.asarray(b2, np.float32)

    if "nc" not in _cache:
        _cache["nc"] = _build()
    nc = _cache["nc"]
    in_maps = _host_inputs(features, sub_mask, obj_mask, W1, b1, W2, b2)
    res = bass_utils.run_bass_kernel_spmd(
        nc, in_maps, core_ids=list(range(N_CORES)), trace=_trace)
    out = np.concatenate([res.results[c]["out"] for c in range(N_CORES)], axis=0)
    if _trace:
        _cache["last_result"] = res
    return out
